# revision 7
# baseline (speedup 1.0000x reference)
"""Multi-head attention (B=2, D=2048, N=1024, H=16) on 8 TRN2 NeuronCores.

Sharding: batch*heads across cores — core c handles batch c//4, heads
4*(c%4) .. 4*(c%4)+3. No collectives.

v3: fully-overlapped schedule + uniform 128-row stationaries.
  - DMA lands weights first, then x^T in seq-window slices; attention for
    head 0 starts as soon as q(h0/h1, queries 0:1024) and k(keys 0:512)
    are projected (~20us in). Remaining projection chains interleave into
    the attention loop's PE slack one matmul at a time; scores for
    iteration t+1 are emitted before PV of iteration t across head
    boundaries, so neither PE nor ScalarE drains at transitions.
  - All matmul stationaries are 128 rows x 128 cols: k tiles live in
    zero-padded per-head regions (kPad — the other parity's rows are
    zero, so contracting the full 128 partitions against the stacked
    q pair adds exact zeros), and v_ext tiles are [v | 1 | 0...] 128
    cols. Uniform row-groups let the PE pull every LDWEIGHTS into the
    background weight buffer behind the running matmul (the v2 profile
    lost ~90ns at every 64<->128 row-group transition), and 128-col
    16-bit weights engage the fast weight load path.
  - PSUM: scores tiles [128,1024] x2 (1-ahead exp pipeline), one PV
    accumulator [128,1024], projection tiles [128,512] x2 = 8 banks.

Per-core math: qT/kT projection in transposed layout [head_dim, seq];
bias as per-partition scalar add on DVE. v in natural layout [seq, 64].
PV accumulates out_ext^T = v_ext^T expS^T; row 64 is the softmax
denominator. No softmax max-subtraction (|S|max ~ 52, exp fits fp32).
Host post-pass divides by the denominator, adds the (linearly separable)
v bias, reshapes to the reference's raw (B,H,D,p)->(B,D,N) layout.

dtypes: f32r projection, f16 q/k, bf16 expS/v_ext.
"""
import sys

sys.path.insert(0, "/opt/trn_rl_repo")

import numpy as np
import ml_dtypes
import concourse.bacc as bacc
import concourse.mybir as mybir
from concourse import tile
from concourse.bass_utils import run_bass_kernel_spmd

B, D, N, H, P = 2, 2048, 1024, 16, 64
NCORES = 8
HPC = 4            # heads per core
KT = 8             # contraction tiles (N / 128)
ST = 4             # seq tiles of 512 for qk projection
JT = 16            # j (key) tiles of 128 per head
F32R = mybir.dt.float32r
F32 = mybir.dt.float32
BF16 = mybir.dt.bfloat16
F16 = mybir.dt.float16
EXP = mybir.ActivationFunctionType.Exp

PJ_DT = F32R       # projection operands (x, W)
QK_DT = F16        # q/k tiles feeding the scores matmul
PV_DT = BF16       # expS + v_ext feeding the PV matmul

# (head, ih) processing order: finish the h0/h1 pair (which only needs
# the m=0/m=2 projection blocks) before h2/h3 (m=1/m=3), so the second
# pair's projections can be interleaved into the first pair's attention.
ATTN_ORDER = [(0, 0), (1, 0), (0, 1), (1, 1),
              (2, 0), (3, 0), (2, 1), (3, 1)]
ITERS = [(h, ih, j) for (h, ih) in ATTN_ORDER for j in range(JT)]

_nc = None


class ChainRunner:
    """Drives projection chains (generators yielding per engine-op) with
    at most one partially-emitted chain at a time, so a later chain's
    first matmul can never deadlock the PE queue against an earlier
    chain's unemitted tail."""

    def __init__(self, make, disc_order):
        self.make = make          # key -> fresh generator
        self.done = set()
        self.cur_key = None
        self.cur_gen = None
        self.disc = list(disc_order)

    def _finish_current(self):
        if self.cur_gen is not None:
            for _ in self.cur_gen:
                pass
            self.done.add(self.cur_key)
            self.cur_key = self.cur_gen = None

    def ensure(self, key):
        if key in self.done:
            return
        if self.cur_key == key:
            self._finish_current()
            return
        self._finish_current()
        for _ in self.make(key):
            pass
        self.done.add(key)

    def drive(self, units):
        while units > 0:
            if self.cur_gen is None:
                while self.disc and self.disc[0] in self.done:
                    self.disc.pop(0)
                if not self.disc:
                    return
                self.cur_key = self.disc.pop(0)
                self.cur_gen = self.make(self.cur_key)
            try:
                next(self.cur_gen)
                units -= 1
            except StopIteration:
                self.done.add(self.cur_key)
                self.cur_key = self.cur_gen = None

    def finish_all(self):
        self._finish_current()
        while self.disc:
            key = self.disc.pop(0)
            if key not in self.done:
                self.ensure(key)


def _build():
    global _nc
    if _nc is not None:
        return _nc
    nc = bacc.Bacc("TRN2", target_bir_lowering=False, debug=False,
                   num_devices=NCORES)
    xt = nc.dram_tensor("xt", [N, D], PJ_DT, kind="ExternalInput").ap()
    wqk = nc.dram_tensor("wqk", [N, 2 * HPC * P], PJ_DT,
                         kind="ExternalInput").ap()
    wv = nc.dram_tensor("wv", [N, HPC * P], PJ_DT, kind="ExternalInput").ap()
    bqk = nc.dram_tensor("bqk", [128, 4], F32, kind="ExternalInput").ap()
    o = nc.dram_tensor("o", [HPC, P + 1, D], F32, kind="ExternalOutput").ap()

    with tile.TileContext(nc) as tc:
        with tc.tile_pool(name="big", bufs=1) as big, \
             tc.tile_pool(name="es", bufs=6) as es, \
             tc.tile_pool(name="obp", bufs=2) as obp:

            # x^T staged s-major: [128, s(4) x k(8) x 512] — a per-s DMA
            # then writes one contiguous 16KB/partition destination block
            # (128 descriptors, ~0.7us issue on the sync queue, vs ~5us
            # for a 2KB-run destination).
            xt_t = big.tile([128, ST * KT * 512], PJ_DT, tag="xt")
            wqk_t = big.tile([128, KT * 512], PJ_DT, tag="wqk")
            wv_t = big.tile([128, KT * 256], PJ_DT, tag="wv")
            bqk_t = big.tile([128, 4], F32, tag="bqk")
            # q in transposed pair layout [128 = 64 even | 64 odd, seq]
            qT = big.tile([128, 2 * D], QK_DT, tag="qT")
            # k in zero-padded per-head regions: head h occupies rows
            # bp..bp+64 of kPad[:, h*D : (h+1)*D]; the other rows are 0.
            kPad = big.tile([128, HPC * D], QK_DT, tag="kPad")
            # v_ext per (j, h): [v(64) | 1 | 0(63)] -> 128-col stationary
            vx = big.tile([128, JT * HPC * 128], PV_DT, tag="vx")

            # DMA order = arrival priority: qk weights, then x^T seq
            # windows s0/s1 (first attention block), then wv, s2, s3.
            nc.sync.dma_start(out=bqk_t[:], in_=bqk)
            nc.sync.dma_start(
                out=wqk_t.rearrange("p (k m) -> p k m", k=KT),
                in_=wqk.rearrange("(k p) m -> p k m", p=128))
            xt_dst = xt_t.rearrange("p (s k d) -> p s k d", s=ST, k=KT)
            xt_src = xt.rearrange("(k p) d -> p k d", p=128)
            for s in (0, 1):
                nc.sync.dma_start(
                    out=xt_dst[:, s],
                    in_=xt_src[:, :, s * 512:(s + 1) * 512])
            nc.sync.dma_start(
                out=wv_t.rearrange("p (k m) -> p k m", k=KT),
                in_=wv.rearrange("(k p) m -> p k m", p=128))
            for s in (2, 3):
                nc.sync.dma_start(
                    out=xt_dst[:, s],
                    in_=xt_src[:, :, s * 512:(s + 1) * 512])

            # zero-init the padded k regions and v_ext tiles; the ones
            # column of v_ext is col 64 of each 128-col block.
            nc.gpsimd.memset(kPad[:], 0.0)
            nc.gpsimd.memset(vx[:], 0.0)
            nc.gpsimd.memset(
                vx.rearrange("p (t c) -> p t c", c=128)[:, :, 64:65], 1.0)

            with tc.tile_pool(name="ps", bufs=2, space="PSUM") as ps, \
                 tc.tile_pool(name="po", bufs=1, space="PSUM") as po, \
                 tc.tile_pool(name="pj", bufs=2, space="PSUM") as pj:

                def qk_chain(m, s):
                    pt = pj.tile([128, 512], F32, tag="pj",
                                 name=f"pq{m}{s}")
                    for k in range(KT):
                        nc.tensor.matmul(
                            pt[:],
                            wqk_t[:, k * 512 + m * 128:
                                  k * 512 + (m + 1) * 128],
                            xt_t[:, s * 4096 + k * 512:
                                 s * 4096 + (k + 1) * 512],
                            start=(k == 0), stop=(k == KT - 1))
                        yield
                    if m < 2:
                        # q pair block: rows 0:64 = even head, 64:128 = odd
                        nc.vector.tensor_scalar_add(
                            qT[:, m * D + s * 512:m * D + (s + 1) * 512],
                            pt[:], bqk_t[:, m:m + 1])
                        yield
                    else:
                        # k block: scatter the two heads into their
                        # zero-padded regions (even head rows 0:64,
                        # odd head rows 64:128).
                        he = 2 * (m - 2)
                        nc.vector.tensor_scalar_add(
                            kPad[0:64, he * D + s * 512:
                                 he * D + (s + 1) * 512],
                            pt[0:64, :], bqk_t[0:64, m:m + 1])
                        yield
                        nc.vector.tensor_scalar_add(
                            kPad[64:128, (he + 1) * D + s * 512:
                                 (he + 1) * D + (s + 1) * 512],
                            pt[64:128, :], bqk_t[64:128, m:m + 1])
                        yield

                def v_chain(j):
                    pt = pj.tile([128, 256], F32, tag="pj", name=f"pv{j}")
                    xo = (j // 4) * 4096 + (j % 4) * 128
                    for k in range(KT):
                        nc.tensor.matmul(
                            pt[:],
                            xt_t[:, xo + k * 512:xo + k * 512 + 128],
                            wv_t[:, k * 256:(k + 1) * 256],
                            start=(k == 0), stop=(k == KT - 1))
                        yield
                    for hh in range(HPC):
                        nc.vector.tensor_copy(
                            vx[:, (j * HPC + hh) * 128:
                               (j * HPC + hh) * 128 + 64],
                            pt[:, hh * 64:(hh + 1) * 64])
                    yield

                def make(key):
                    if key[0] == "qk":
                        return qk_chain(key[1], key[2])
                    return v_chain(key[1])

                # discretionary pre-drive order = first-need order
                pre = [("qk", 0, 0), ("qk", 0, 1), ("qk", 2, 0)]
                seen = set(pre)
                disc = []
                for (h, ih, j) in ITERS:
                    mq, mk = (0, 2) if h < 2 else (1, 3)
                    for key in [("v", j), ("qk", mk, j // 4),
                                ("qk", mq, 2 * ih), ("qk", mq, 2 * ih + 1)]:
                        if key not in seen:
                            disc.append(key)
                            seen.add(key)
                runner = ChainRunner(make, disc)
                for key in pre:
                    runner.ensure(key)

                def emit_scores(h, ih, j):
                    st = ps.tile([128, 1024], F32, tag="ps", name="st")
                    qoff = (h // 2) * D + ih * 1024
                    for i2 in range(2):
                        nc.tensor.matmul(
                            st[:, i2 * 512:(i2 + 1) * 512],
                            kPad[:, h * D + j * 128:h * D + (j + 1) * 128],
                            qT[:, qoff + i2 * 512:qoff + (i2 + 1) * 512],
                            start=True, stop=True)
                    return st

                sts = {0: emit_scores(*ITERS[0])}
                ot = None
                for t, (h, ih, j) in enumerate(ITERS):
                    if t + 1 < len(ITERS):
                        hn, ihn, jn = ITERS[t + 1]
                        mq, mk = (0, 2) if hn < 2 else (1, 3)
                        runner.ensure(("qk", mq, 2 * ihn))
                        runner.ensure(("qk", mq, 2 * ihn + 1))
                        runner.ensure(("qk", mk, jn // 4))
                        sts[t + 1] = emit_scores(hn, ihn, jn)
                    et = es.tile([128, 1024], PV_DT, tag="et", name="et")
                    nc.scalar.activation(et[:], sts.pop(t)[:], EXP)
                    runner.ensure(("v", j))
                    if t >= JT:
                        runner.drive(1)
                    if j == 0:
                        ot = po.tile([128, 1024], F32, tag="po", name="ot")
                    for i2 in range(2):
                        nc.tensor.matmul(
                            ot[:, i2 * 512:(i2 + 1) * 512],
                            vx[:, (j * HPC + h) * 128:
                               (j * HPC + h + 1) * 128],
                            et[:, i2 * 512:(i2 + 1) * 512],
                            start=(j == 0), stop=(j == JT - 1))
                    if j == JT - 1:
                        ob = obp.tile([P + 1, 1024], F32, tag="ob",
                                      name="ob")
                        nc.vector.tensor_copy(ob[:], ot[0:P + 1, :])
                        nc.sync.dma_start(
                            out=o.rearrange("h p d -> (h p) d")[
                                h * 65:(h + 1) * 65,
                                ih * 1024:(ih + 1) * 1024],
                            in_=ob[:])
                runner.finish_all()
    nc.compile()
    _nc = nc
    return nc


def _np_dt(dt):
    if dt == BF16:
        return ml_dtypes.bfloat16
    if dt == mybir.dt.float16:
        return np.float16
    return np.float32


def _shard_inputs(x, W_qkv, b_qkv):
    pj = _np_dt(PJ_DT)
    in_maps = []
    for c in range(NCORES):
        b = c // 4
        h0 = HPC * (c % 4)
        xT = np.ascontiguousarray(x[b].T).astype(pj)
        wq = W_qkv[:, h0 * P:(h0 + HPC) * P]
        wk = W_qkv[:, N + h0 * P:N + (h0 + HPC) * P]
        wqk = np.ascontiguousarray(np.concatenate([wq, wk], axis=1)).astype(pj)
        wv = np.ascontiguousarray(
            W_qkv[:, 2 * N + h0 * P:2 * N + (h0 + HPC) * P]).astype(pj)
        bq = b_qkv[h0 * P:(h0 + HPC) * P]
        bk = b_qkv[N + h0 * P:N + (h0 + HPC) * P]
        bqk = np.ascontiguousarray(
            np.concatenate([bq, bk]).reshape(4, 128).T).astype(np.float32)
        in_maps.append({"xt": xT, "wqk": wqk, "wv": wv, "bqk": bqk})
    return in_maps


def _assemble(results, b_qkv):
    out = np.empty((B, D, N), dtype=np.float32)
    for c in range(NCORES):
        b = c // 4
        h0 = HPC * (c % 4)
        oe = results[c]["o"]                      # (4, 65, 2048)
        att = oe[:, :P, :] / oe[:, P:P + 1, :]    # (4, 64, 2048)
        att = np.transpose(att, (0, 2, 1))        # (4, 2048, 64)
        for hl in range(HPC):
            h = h0 + hl
            bv = b_qkv[2 * N + h * P:2 * N + (h + 1) * P]
            out[b, h * 128:(h + 1) * 128, :] = \
                (att[hl] + bv[None, :]).reshape(128, N)
    return out


def _forward(in_maps, **kwargs):
    nc = _build()
    return run_bass_kernel_spmd(nc, in_maps, core_ids=list(range(NCORES)),
                                **kwargs)


def kernel(x, W_qkv, b_qkv):
    x = np.asarray(x, dtype=np.float32)
    W_qkv = np.asarray(W_qkv, dtype=np.float32)
    b_qkv = np.asarray(b_qkv, dtype=np.float32)
    in_maps = _shard_inputs(x, W_qkv, b_qkv)
    res = _forward(in_maps)
    return _assemble(res.results, b_qkv)
